# revision 13
# baseline (speedup 1.0000x reference)
"""Trainium2 Bass kernel for nn_DES_PSP_Model (LSTM encoder + CNN + AR decoder).

Sharding: pure data parallel, batch 128 -> 8 cores x 16.

v2: latency-chain-optimized.
- Encoder: 5-layer LSTM over T=256 as a time wavefront; the 16-batch is split
  into NG=2 phase groups of 8 whose dependency chains interleave on the
  engines (effective tick ~ chain/2). Per tick+group: bias via K=5 const
  matmul, x via K=1 matmul reading xw directly, layer0 recurrence K=64
  (base 64), layers 1-4 K-stacked [in;rec] K=128. Cell math in the all-tanh
  form (states stored 2h / 2c); m2 and the H shift-copy run on Pool (gpsimd)
  so the DVE chain is m1 -> C' -> H'.
- CNN: conv0+avgpool folded into one 4x4/stride-2 im2col matmul (K=16);
  conv1-7 as tap-pair K=128 matmuls on a partition-duplicated activation
  tile; all CNN work is interleaved into the encoder tick loop.
- Decoder: 14 steps x 5 layers, 2 phase groups; the fc layer is folded into
  the next step's layer-0 input matmul (rank-1 K=64 on h^4), so fc+output
  never sits on the autoregressive chain; biases via K=1 ones-matmuls so
  each cell needs a single ACT.
"""
import os
import sys
import numpy as np
from contextlib import ExitStack

sys.path.insert(0, "/opt/trn_rl_repo")
os.environ.setdefault("JAX_PLATFORMS", "axon")

import ml_dtypes  # noqa: E402

BF = ml_dtypes.bfloat16

B, T, HID, L, PS = 128, 256, 64, 5, 14
ALPHA = 0.2
CNN_LAYERS = 8
NCORES = 8
BP = B // NCORES          # 16 batch per core
NG = int(os.environ.get("BASSK_NG", 2))   # phase groups
GW = BP // NG             # batch per group
W5 = L * BP               # 80
GW5 = L * GW
IMG = 32
PM = 16
PPAD = PM + 2             # 18
PIMG = PPAD * PPAD        # 324

# pytorch gate rows: i[0:64] f[64:128] g[128:192] o[192:256]
_PERM_A = np.r_[64:128, 0:64]      # chunkA = [f; i]
_PERM_B = np.r_[192:256, 128:192]  # chunkB = [o; g]

# ---- lw column-block layout (128 cols per block) ----
# top half (rows 64:128) tenant / small tenants at partition 0 only
# (HW quirk: mixing tile_position rows 32 and 0 within one PSUM
#  accumulation group faults the PE, so every K=1 bias row sits at p0)
NBLK = 32
FC_COL = NBLK * 128               # single column, rows 64:128
LW_COLS = FC_COL + 1


def _blk(i):
    return slice(i * 128, (i + 1) * 128)


# block index helpers (c = chunk 0/1)
def _b_encl0(c):      return c          # rows64: ENC_L0REC  | p0: ENC_BIAS(5)
def _b_encx(c):       return 2 + c      # p0: ENC_X(1)       | rows64: DEC_FCIN
def _b_decrec(l, c):  return 4 + 2 * l + c   # rows64: DEC_REC[l]
# p0 row tenants: blk 4+c DEC_X; bias rows via _b_decbias below
def _b_decin(l, c):   return 14 + 2 * (l - 1) + c  # l in 1..4, rows64
def _b_kst(l, c):     return 22 + 2 * (l - 1) + c  # l in 1..4, full 128


def _b_decbias(l, c, k0=False):
    # p0 single-row bias slots: step-0 l0 -> blk12/13, l0 (k>=1) -> blk14/15,
    # l=1..3 -> blk16..21, l=4 -> blk30/31
    if l == 0:
        return (12 + c) if k0 else (14 + c)
    if l <= 3:
        return 14 + 2 * l + c
    return 30 + c


def _gate_row_scale():
    sA = np.ones(128, np.float32)
    sB = np.ones(128, np.float32)
    sB[64:128] = 2.0
    return sA, sB


def _chunk(W, perm, rowscale):
    # W: [4H, K] -> permuted+scaled chunk [128, K]
    return W[perm] * rowscale[:, None]


def prep_host(inputs):
    """Build per-core input maps (list of dicts of np arrays)."""
    x = np.asarray(inputs["x"], np.float32)
    y = np.asarray(inputs["y"], np.float32)
    f32 = lambda a: np.asarray(a, np.float32)
    enc_Wih0, enc_Wih = f32(inputs["enc_Wih0"]), f32(inputs["enc_Wih"])
    enc_Whh, enc_b = f32(inputs["enc_Whh"]), f32(inputs["enc_b"])
    dec_Wih0, dec_Wih = f32(inputs["dec_Wih0"]), f32(inputs["dec_Wih"])
    dec_Whh, dec_b = f32(inputs["dec_Whh"]), f32(inputs["dec_b"])
    fc_W, fc_b = f32(inputs["fc_W"]), f32(inputs["fc_b"])
    conv0_W, conv0_b = f32(inputs["conv0_W"]), f32(inputs["conv0_b"])
    convs_W, convs_b = f32(inputs["convs_W"]), f32(inputs["convs_b"])

    sA, sB = _gate_row_scale()
    perms = [_PERM_A, _PERM_B]
    scales = [sA, sB]

    lw = np.zeros((128, LW_COLS), np.float32)

    def cvec(v, c):  # [256] -> permuted/scaled [128]
        return _chunk(v[:, None], perms[c], scales[c])[:, 0]

    for c in range(2):
        p, s = perms[c], scales[c]
        bb = _blk(_b_encl0(c))
        # ENC_BIAS rows 0:5
        for j in range(L):
            lw[j, bb] = cvec(enc_b[j], c)
        # ENC_L0REC rows 64:128
        lw[64:128, bb] = (_chunk(enc_Whh[0], p, s) * 0.5).T
        bx = _blk(_b_encx(c))
        # ENC_X row 0 (x input: no 0.5)
        lw[0, bx] = _chunk(enc_Wih0, p, s)[:, 0]
        # DEC_FCIN rows 64:128 : outer(0.5*fc_W, dec_Wih0_chunk)
        lw[64:128, bx] = np.outer(0.5 * fc_W[0], _chunk(dec_Wih0, p, s)[:, 0])
        # ENC_KST layers 1..4
        for l in range(1, L):
            ci = _chunk(enc_Wih[l - 1], p, s) * 0.5
            ch = _chunk(enc_Whh[l], p, s) * 0.5
            lw[:, _blk(_b_kst(l, c))] = np.concatenate([ci.T, ch.T], axis=0)
        # DEC_REC blocks + small tenants
        for l in range(L):
            br = _blk(_b_decrec(l, c))
            lw[64:128, br] = (_chunk(dec_Whh[l], p, s) * 0.5).T
        lw[0, _blk(4 + c)] = _chunk(dec_Wih0, p, s)[:, 0]          # DEC_X
        lw[0, _blk(_b_decbias(0, c, k0=True))] = cvec(dec_b[0], c)
        lw[0, _blk(_b_decbias(0, c))] = cvec(
            dec_b[0] + dec_Wih0[:, 0] * fc_b[0], c)
        for l in range(1, L):
            lw[0, _blk(_b_decbias(l, c))] = cvec(dec_b[l], c)
        # DEC_IN layers 1..4
        for l in range(1, L):
            lw[64:128, _blk(_b_decin(l, c))] = (
                _chunk(dec_Wih[l - 1], p, s) * 0.5).T
    lw[64:128, FC_COL] = 0.5 * fc_W[0]
    lw = lw.astype(BF)

    # ---- cnnw: conv1-7 tap-pair stationaries + conv0 4x4 folded ----
    cb = []
    for i in range(CNN_LAYERS - 1):
        for pp in range(6):
            blk = np.zeros((128, 64), np.float32)
            if pp < 3:
                dy = pp - 1
                blk[0:64] = convs_W[i, :, :, dy + 1, 0].T
                blk[64:128] = convs_W[i, :, :, dy + 1, 1].T
            else:
                dy = pp - 4
                blk[0:64] = convs_W[i, :, :, dy + 1, 2].T
            cb.append(blk)
    # conv0 folded with avgpool: 4x4 stride-2 kernel, 16 taps
    W4 = np.zeros((16, 64), np.float32)
    for u in range(4):
        for v in range(4):
            acc = np.zeros(64, np.float32)
            for a in range(2):
                for b2 in range(2):
                    if 0 <= u - a <= 2 and 0 <= v - b2 <= 2:
                        acc += 0.25 * conv0_W[:, 0, u - a, v - b2]
            W4[u * 4 + v] = acc
    c0blk = np.zeros((128, 64), np.float32)
    c0blk[0:16] = W4
    cb.append(c0blk)
    cnnw = np.concatenate(cb, axis=1).astype(BF)   # [128, 42*64+64=2752]
    C0_COL = (CNN_LAYERS - 1) * 6 * 64             # 2688

    # ---- indc: [5, GW5] layer indicators ----
    indc = np.zeros((5, GW5), np.float32)
    for j in range(L):
        indc[j, j * GW:(j + 1) * GW] = 1.0
    indc = indc.astype(BF)

    # ---- ones row (for K=1 bias matmuls at partition 0) ----
    ones = np.ones((1, GW), np.float32).astype(BF)

    # ---- misc f32 [64, 16]: conv/fc biases ----
    misc = np.zeros((64, 16), np.float32)
    misc[:, 0] = conv0_b
    for i in range(CNN_LAYERS - 1):
        misc[:, 1 + i] = convs_b[i]
    misc[0, 8] = fc_b[0]

    # ---- per-core tensors ----
    ypad = np.pad(y[:, 0], ((0, 0), (1, 1), (1, 1)))  # [B, 34, 34]
    in_maps = []
    for cidx in range(NCORES):
        sl = slice(cidx * BP, (cidx + 1) * BP)
        xs = x[sl, :, 0]  # [BP, T]
        xtm = np.ascontiguousarray(xs.T).reshape(1, T * BP).astype(BF)
        yp = ypad[sl]  # [BP, 34, 34]
        # conv0 4x4/stride2 im2col: [16, BP*256]
        yim = np.zeros((16, BP, PM, PM), np.float32)
        for u in range(4):
            for v in range(4):
                yim[u * 4 + v] = yp[:, u:u + 32:2, v:v + 32:2]
        yim = yim.reshape(16, BP * PM * PM).astype(BF)
        in_maps.append(dict(
            lstmw=lw, cnnw=cnnw, indc=indc, ones=ones, misc=misc,
            x=xtm, yim=yim,
        ))
    return in_maps, C0_COL


# ----------------------------------------------------------------------------
# device program
# ----------------------------------------------------------------------------

_CACHE = {}


def build_program(C0_COL):
    import concourse.bass as bass  # noqa: F401
    import concourse.tile as tile
    from concourse import bacc, mybir

    F32 = mybir.dt.float32
    BF16 = mybir.dt.bfloat16
    AF = mybir.ActivationFunctionType
    OP = mybir.AluOpType

    TICKS = int(os.environ.get("BASSK_TICKS", T + L - 1))  # 260
    DSTEPS = int(os.environ.get("BASSK_DSTEPS", PS))
    DO_CNN = int(os.environ.get("BASSK_CNN", 1))
    NCONV = int(os.environ.get("BASSK_NCONV", CNN_LAYERS))
    CNN_EVERY = int(os.environ.get("BASSK_CNN_EVERY", 7))

    nc = bacc.Bacc("TRN2", target_bir_lowering=False, debug=False,
                   num_devices=NCORES)
    d_lstmw = nc.dram_tensor("lstmw", [128, LW_COLS], BF16,
                             kind="ExternalInput").ap()
    d_cnnw = nc.dram_tensor("cnnw", [128, 2752], BF16,
                            kind="ExternalInput").ap()
    d_indc = nc.dram_tensor("indc", [5, GW5], BF16, kind="ExternalInput").ap()
    d_ones = nc.dram_tensor("ones", [1, GW], BF16, kind="ExternalInput").ap()
    d_misc = nc.dram_tensor("misc", [64, 16], F32, kind="ExternalInput").ap()
    d_x = nc.dram_tensor("x", [1, T * BP], BF16, kind="ExternalInput").ap()
    d_yim = nc.dram_tensor("yim", [16, BP * PM * PM], BF16,
                           kind="ExternalInput").ap()
    d_out = nc.dram_tensor("out", [1, PS * BP], F32, kind="ExternalOutput").ap()

    with tile.TileContext(nc) as tc:
        with ExitStack() as ctx:
            const = ctx.enter_context(tc.tile_pool(name="const", bufs=1))
            state = ctx.enter_context(tc.tile_pool(name="state", bufs=1))
            spool = ctx.enter_context(tc.tile_pool(name="spool", bufs=2))
            mpool = ctx.enter_context(tc.tile_pool(name="mpool", bufs=2))
            dpool = ctx.enter_context(tc.tile_pool(name="dpool", bufs=2))
            eps = ctx.enter_context(
                tc.tile_pool(name="eps", bufs=NG, space="PSUM"))
            cps = ctx.enter_context(tc.tile_pool(name="cps", bufs=2,
                                                 space="PSUM"))
            dps = ctx.enter_context(tc.tile_pool(name="dps", bufs=3,
                                                 space="PSUM"))
            fps = ctx.enter_context(tc.tile_pool(name="fps", bufs=1,
                                                 space="PSUM"))

            # ---- constants ----
            lw = const.tile([128, LW_COLS], BF16, tag="lw", name="lw")
            nc.sync.dma_start(lw[:], d_lstmw)
            cw = const.tile([128, 2752], BF16, tag="cw", name="cw")
            if DO_CNN:
                nc.sync.dma_start(cw[:], d_cnnw)
            indct = const.tile([5, GW5], BF16, tag="indct", name="indct")
            nc.sync.dma_start(indct[:], d_indc)
            onest = const.tile([1, GW], BF16, tag="onest", name="onest")
            nc.sync.dma_start(onest[:], d_ones)
            misct = const.tile([64, 16], F32, tag="misct", name="misct")
            nc.sync.dma_start(misct[:], d_misc)
            xw = const.tile([1, T * BP], BF16, tag="xw", name="xw")
            nc.sync.dma_start(xw[:], d_x)
            yimt = const.tile([16, BP * PM * PM], BF16, tag="yimt",
                              name="yimt")
            if DO_CNN:
                nc.sync.dma_start(yimt[:], d_yim)

            # ---- persistent state ----
            Ht = state.tile([128, W5], BF16, tag="H", name="H")
            Ct = state.tile([64, W5], F32, tag="C", name="C")
            nc.gpsimd.memset(Ht[:], 0.0)
            nc.gpsimd.memset(Ct[:], 0.0)
            z2a = state.tile([128, BP * PIMG], BF16, tag="z2a", name="z2a")
            z2b = state.tile([128, BP * PIMG], BF16, tag="z2b", name="z2b")
            if DO_CNN:
                nc.gpsimd.memset(z2a[:], 0.0)
                nc.gpsimd.memset(z2b[:], 0.0)
            feat = state.tile([64, BP], F32, tag="feat", name="feat")
            feat2 = state.tile([128, BP], BF16, tag="feat2", name="feat2")
            outt = state.tile([1, PS * BP], F32, tag="outt", name="outt")
            if DSTEPS == 0 or int(os.environ.get("BASSK_NOFC", 0)):
                nc.gpsimd.memset(outt[:], 0.0)
            if not DO_CNN:
                nc.gpsimd.memset(feat2[:], 0.0)

            Htv = Ht[:].rearrange("p (l b) -> p l b", l=L)
            Ctv = Ct[:].rearrange("p (l b) -> p l b", l=L)
            indcv = indct[:].rearrange("p (l b) -> p l b", l=L)

            # ============ CNN thunks (interleaved into encoder) ============
            cnn_thunks = []
            if DO_CNN:
                z1v = z2a[:].rearrange("p (i r c) -> p i r c", i=BP, r=PPAD)

                def conv0_chunk(n):
                    def f():
                        pc = cps.tile([64, 512], F32, tag="cpg", name="cpg")
                        nc.tensor.matmul(
                            pc[:], cw[0:16, C0_COL:C0_COL + 64],
                            yimt[0:16, n * 512:(n + 1) * 512],
                            start=True, stop=True)
                        nc.scalar.activation(
                            z1v[0:64, 2 * n:2 * n + 2, 1:17, 1:17],
                            pc[:].rearrange("p (i r c) -> p i r c", i=2, r=16),
                            AF.Identity, bias=misct[0:64, 0:1])
                        nc.gpsimd.tensor_copy(
                            z1v[64:128, 2 * n:2 * n + 2, 1:17, 0:16],
                            z1v[0:64, 2 * n:2 * n + 2, 1:17, 1:17])
                    return f

                for n in range(BP // 2):
                    cnn_thunks.append(conv0_chunk(n))

                def conv_group(i, n, zin, zout):
                    def f():
                        ziv = zin[:].rearrange("p (i r c) -> p i r c",
                                               i=BP, r=PPAD)
                        zov = zout[:].rearrange("p (i r c) -> p i r c",
                                                i=BP, r=PPAD)
                        i0 = 2 * n
                        pc = cps.tile([64, 512], F32, tag="cpg", name="cpg")
                        for pp in range(6):
                            dy = (pp - 1) if pp < 3 else (pp - 4)
                            c0_ = 0 if pp < 3 else 2
                            st_ = cw[:, (i - 1) * 384 + pp * 64:
                                     (i - 1) * 384 + pp * 64 + 64]
                            rhs = ziv[:, i0:i0 + 2, 1 + dy:17 + dy,
                                      c0_:c0_ + 16]
                            nc.tensor.matmul(pc[:], st_, rhs,
                                             start=(pp == 0), stop=(pp == 5))
                        nc.scalar.activation(
                            zov[0:64, i0:i0 + 2, 1:17, 1:17],
                            pc[:].rearrange("p (i r c) -> p i r c", i=2, r=16),
                            AF.Relu, bias=misct[0:64, i:i + 1])
                        if i < CNN_LAYERS - 1:
                            nc.gpsimd.tensor_copy(
                                zov[64:128, i0:i0 + 2, 1:17, 0:16],
                                zov[0:64, i0:i0 + 2, 1:17, 1:17])
                    return f

                zin, zout = z2a, z2b
                for i in range(1, NCONV):
                    for n in range(BP // 2):
                        cnn_thunks.append(conv_group(i, n, zin, zout))
                    zin, zout = zout, zin

                def gap_thunk(zfin):
                    def f():
                        zfv = zfin[:].rearrange("p (i r c) -> p i r c",
                                                i=BP, r=PPAD)
                        for j in range(BP):
                            nc.vector.tensor_reduce(
                                feat[:, j:j + 1], zfv[0:64, j, 1:17, 1:17],
                                axis=mybir.AxisListType.XY, op=OP.add)
                        nc.vector.tensor_copy(feat2[64:128, :], feat[:])
                    return f

                cnn_thunks.append(gap_thunk(zin))
            cnn_iter = iter(cnn_thunks)
            sub_idx = 0

            # =============== encoder wavefront ===============
            for s in range(TICKS):
                lmin = max(0, s - (T - 1))
                lmax = min(L - 1, s)
                sl = slice(lmin, lmax + 1)
                for g in range(NG):
                    gsl = slice(g * GW, (g + 1) * GW)
                    pg = eps.tile([128, 2 * GW5], F32, tag="epg", name="epg")
                    pgv = pg[:].rearrange("p (c l b) -> p c l b", c=2, l=L)
                    for c in range(2):
                        nc.tensor.matmul(
                            pgv[:, c, sl, :], lw[0:5, _blk(_b_encl0(c))],
                            indcv[:, sl, :], start=True, stop=False)
                        if lmin == 0:
                            nc.tensor.matmul(
                                pgv[:, c, 0, :], lw[0:1, _blk(_b_encx(c))],
                                xw[0:1, s * BP + g * GW:s * BP + g * GW + GW],
                                start=False, stop=False)
                            nc.tensor.matmul(
                                pgv[:, c, 0, :], lw[64:128, _blk(_b_encl0(c))],
                                Htv[64:128, 0, gsl],
                                start=False, stop=(lmax == 0))
                        for l in range(max(1, lmin), lmax + 1):
                            nc.tensor.matmul(
                                pgv[:, c, l, :], lw[:, _blk(_b_kst(l, c))],
                                Htv[:, l, gsl],
                                start=False, stop=(l == lmax))
                    st = spool.tile([128, 2 * GW5], F32, tag=f"st{g}",
                                    name="st")
                    stv = st[:].rearrange("p (c l b) -> p c l b", c=2, l=L)
                    nc.scalar.activation(stv[:, :, sl, :], pgv[:, :, sl, :],
                                         AF.Tanh, scale=0.5)
                    m1 = mpool.tile([64, GW5], F32, tag=f"m1{g}", name="m1")
                    m2 = mpool.tile([64, GW5], F32, tag=f"m2{g}", name="m2")
                    tcn = mpool.tile([64, GW5], F32, tag=f"tc{g}", name="tcn")
                    m1v = m1[:].rearrange("p (l b) -> p l b", l=L)
                    m2v = m2[:].rearrange("p (l b) -> p l b", l=L)
                    tcv = tcn[:].rearrange("p (l b) -> p l b", l=L)
                    nc.vector.scalar_tensor_tensor(
                        m1v[:, sl, :], stv[0:64, 0, sl, :], 1.0,
                        Ctv[:, sl, gsl], op0=OP.add, op1=OP.mult)
                    nc.vector.scalar_tensor_tensor(
                        m2v[:, sl, :], stv[64:128, 0, sl, :], 1.0,
                        stv[64:128, 1, sl, :], op0=OP.add, op1=OP.mult)
                    nc.vector.scalar_tensor_tensor(
                        Ctv[:, sl, gsl], m1v[:, sl, :], 0.5,
                        m2v[:, sl, :], op0=OP.mult, op1=OP.add)
                    nc.scalar.activation(tcv[:, sl, :], Ctv[:, sl, gsl],
                                         AF.Tanh, scale=0.5)
                    nc.vector.scalar_tensor_tensor(
                        Htv[64:128, sl, gsl], stv[0:64, 1, sl, :], 1.0,
                        tcv[:, sl, :], op0=OP.add, op1=OP.mult)
                    # shift-copy for next tick: top[l] = bot[l-1] (on Pool)
                    if s + 1 < TICKS:
                        nlmax = min(L - 1, s + 1)
                        a = max(1, max(0, s + 1 - (T - 1)))
                        if nlmax >= 1:
                            nc.gpsimd.tensor_copy(
                                Htv[0:64, a:nlmax + 1, gsl],
                                Htv[64:128, a - 1:nlmax, gsl])
                    sub_idx += 1
                    if sub_idx % CNN_EVERY == 0:
                        th = next(cnn_iter, None)
                        if th is not None:
                            th()
            for th in cnn_iter:
                th()

            # =============== fuse -> decoder init ===============
            kf = 2.0 * ALPHA / 256.0
            for l in range(L):
                nc.vector.scalar_tensor_tensor(
                    Htv[64:128, l, :], feat2[64:128, :], kf,
                    Htv[64:128, l, :], op0=OP.mult, op1=OP.add)

            # =============== decoder ===============
            for k in range(DSTEPS):
                for g in range(NG):
                    gsl = slice(g * GW, (g + 1) * GW)
                    for l in range(L):
                        pd = dps.tile([128, 2 * GW], F32, tag="dpg",
                                      name="dpg")
                        pdv = pd[:].rearrange("p (c b) -> p c b", c=2)
                        for c in range(2):
                            # group order: p0 matmuls first, then p64
                            bias_ap = lw[0:1, _blk(_b_decbias(l, c, k == 0))]
                            nc.tensor.matmul(pdv[:, c, :], bias_ap,
                                             onest[0:1, :],
                                             start=True, stop=False)
                            if l == 0 and k == 0:
                                nc.tensor.matmul(
                                    pdv[:, c, :], lw[0:1, _blk(4 + c)],
                                    xw[0:1, (T - 1) * BP + g * GW:
                                       (T - 1) * BP + g * GW + GW],
                                    start=False, stop=False)
                            nc.tensor.matmul(
                                pdv[:, c, :], lw[64:128, _blk(_b_decrec(l, c))],
                                Htv[64:128, l, gsl], start=False,
                                stop=(l == 0 and k == 0))
                            if l == 0:
                                if k > 0:
                                    nc.tensor.matmul(
                                        pdv[:, c, :],
                                        lw[64:128, _blk(_b_encx(c))],
                                        Htv[64:128, L - 1, gsl],
                                        start=False, stop=True)
                            else:
                                nc.tensor.matmul(
                                    pdv[:, c, :],
                                    lw[64:128, _blk(_b_decin(l, c))],
                                    Htv[64:128, l - 1, gsl],
                                    start=False, stop=True)
                        sd = dpool.tile([128, 2 * GW], F32, tag=f"sd{g}",
                                        name="sd")
                        sdv = sd[:].rearrange("p (c b) -> p c b", c=2)
                        nc.scalar.activation(sd[:], pd[:], AF.Tanh, scale=0.5)
                        dm1 = dpool.tile([64, GW], F32, tag=f"dm1{g}",
                                         name="dm1")
                        dm2 = dpool.tile([64, GW], F32, tag=f"dm2{g}",
                                         name="dm2")
                        dtc = dpool.tile([64, GW], F32, tag=f"dtc{g}",
                                         name="dtc")
                        nc.vector.scalar_tensor_tensor(
                            dm1[:], sdv[0:64, 0, :], 1.0, Ctv[:, l, gsl],
                            op0=OP.add, op1=OP.mult)
                        nc.vector.scalar_tensor_tensor(
                            dm2[:], sdv[64:128, 0, :], 1.0,
                            sdv[64:128, 1, :], op0=OP.add, op1=OP.mult)
                        nc.vector.scalar_tensor_tensor(
                            Ctv[:, l, gsl], dm1[:], 0.5, dm2[:],
                            op0=OP.mult, op1=OP.add)
                        nc.scalar.activation(dtc[:], Ctv[:, l, gsl], AF.Tanh,
                                             scale=0.5)
                        nc.vector.scalar_tensor_tensor(
                            Htv[64:128, l, gsl], sdv[0:64, 1, :], 1.0,
                            dtc[:], op0=OP.add, op1=OP.mult)
                    # fc + output (off the AR chain)
                    if int(os.environ.get("BASSK_NOFC", 0)) == 0:
                        pf = fps.tile([1, GW], F32, tag="fpg", name="fpg")
                        nc.tensor.matmul(pf[:], lw[64:128, FC_COL:FC_COL + 1],
                                         Htv[64:128, L - 1, gsl],
                                         start=True, stop=True)
                        nc.scalar.activation(
                            outt[0:1, k * BP + g * GW:k * BP + (g + 1) * GW],
                            pf[:], AF.Identity, bias=misct[0:1, 8:9])

            nc.sync.dma_start(d_out, outt[:])

    nc.compile()
    return nc


def kernel(**inputs) -> np.ndarray:
    from concourse.bass_utils import run_bass_kernel_spmd
    in_maps, c0col = prep_host(inputs)
    if "nc" not in _CACHE:
        _CACHE["nc"] = build_program(c0col)
    nc = _CACHE["nc"]
    res = run_bass_kernel_spmd(nc, in_maps, list(range(NCORES)))
    outs = []
    for c in range(NCORES):
        o = np.asarray(res.results[c]["out"], np.float32).reshape(PS, BP)
        outs.append(o.T[:, :, None])  # [BP, PS, 1]
    return np.concatenate(outs, axis=0)


# revision 14
# speedup vs baseline: 1.2078x; 1.2078x over previous
"""Trainium2 Bass kernel for nn_DES_PSP_Model (LSTM encoder + CNN + AR decoder).

Sharding: pure data parallel, batch 128 -> 8 cores x 16.

v2: latency-chain-optimized.
- Encoder: 5-layer LSTM over T=256 as a time wavefront; the 16-batch is split
  into NG=2 phase groups of 8 whose dependency chains interleave on the
  engines (effective tick ~ chain/2). Per tick+group: bias via K=5 const
  matmul, x via K=1 matmul reading xw directly, layer0 recurrence K=64
  (base 64), layers 1-4 K-stacked [in;rec] K=128. Cell math in the all-tanh
  form (states stored 2h / 2c); m2 and the H shift-copy run on Pool (gpsimd)
  so the DVE chain is m1 -> C' -> H'.
- CNN: conv0+avgpool folded into one 4x4/stride-2 im2col matmul (K=16);
  conv1-7 as tap-pair K=128 matmuls on a partition-duplicated activation
  tile; all CNN work is interleaved into the encoder tick loop.
- Decoder: 14 steps x 5 layers, 2 phase groups; the fc layer is folded into
  the next step's layer-0 input matmul (rank-1 K=64 on h^4), so fc+output
  never sits on the autoregressive chain; biases via K=1 ones-matmuls so
  each cell needs a single ACT.
"""
import os
import sys
import numpy as np
from contextlib import ExitStack

sys.path.insert(0, "/opt/trn_rl_repo")
os.environ.setdefault("JAX_PLATFORMS", "axon")

import ml_dtypes  # noqa: E402

BF = ml_dtypes.bfloat16

B, T, HID, L, PS = 128, 256, 64, 5, 14
ALPHA = 0.2
CNN_LAYERS = 8
NCORES = 8
BP = B // NCORES          # 16 batch per core
NG = int(os.environ.get("BASSK_NG", 1))   # phase groups
GW = BP // NG             # batch per group
W5 = L * BP               # 80
GW5 = L * GW
IMG = 32
PM = 16
PPAD = PM + 2             # 18
PIMG = PPAD * PPAD        # 324

# pytorch gate rows: i[0:64] f[64:128] g[128:192] o[192:256]
_PERM_A = np.r_[64:128, 0:64]      # chunkA = [f; i]
_PERM_B = np.r_[192:256, 128:192]  # chunkB = [o; g]

# ---- lw column-block layout (128 cols per block) ----
# top half (rows 64:128) tenant / small tenants at partition 0 only
# (HW quirk: mixing tile_position rows 32 and 0 within one PSUM
#  accumulation group faults the PE, so every K=1 bias row sits at p0)
NBLK = 32
FC_COL = NBLK * 128               # single column, rows 64:128
LW_COLS = FC_COL + 1


def _blk(i):
    return slice(i * 128, (i + 1) * 128)


# block index helpers (c = chunk 0/1)
def _b_encl0(c):      return c          # rows64: ENC_L0REC  | p0: ENC_BIAS(5)
def _b_encx(c):       return 2 + c      # p0: ENC_X(1)       | rows64: DEC_FCIN
def _b_decrec(l, c):  return 4 + 2 * l + c   # rows64: DEC_REC[l]
# p0 row tenants: blk 4+c DEC_X; bias rows via _b_decbias below
def _b_decin(l, c):   return 14 + 2 * (l - 1) + c  # l in 1..4, rows64
def _b_kst(l, c):     return 22 + 2 * (l - 1) + c  # l in 1..4, full 128


def _b_decbias(l, c, k0=False):
    # p0 single-row bias slots: step-0 l0 -> blk12/13, l0 (k>=1) -> blk14/15,
    # l=1..3 -> blk16..21, l=4 -> blk30/31
    if l == 0:
        return (12 + c) if k0 else (14 + c)
    if l <= 3:
        return 14 + 2 * l + c
    return 30 + c


def _gate_row_scale():
    sA = np.ones(128, np.float32)
    sB = np.ones(128, np.float32)
    sB[64:128] = 2.0
    return sA, sB


def _chunk(W, perm, rowscale):
    # W: [4H, K] -> permuted+scaled chunk [128, K]
    return W[perm] * rowscale[:, None]


def prep_host(inputs):
    """Build per-core input maps (list of dicts of np arrays)."""
    x = np.asarray(inputs["x"], np.float32)
    y = np.asarray(inputs["y"], np.float32)
    f32 = lambda a: np.asarray(a, np.float32)
    enc_Wih0, enc_Wih = f32(inputs["enc_Wih0"]), f32(inputs["enc_Wih"])
    enc_Whh, enc_b = f32(inputs["enc_Whh"]), f32(inputs["enc_b"])
    dec_Wih0, dec_Wih = f32(inputs["dec_Wih0"]), f32(inputs["dec_Wih"])
    dec_Whh, dec_b = f32(inputs["dec_Whh"]), f32(inputs["dec_b"])
    fc_W, fc_b = f32(inputs["fc_W"]), f32(inputs["fc_b"])
    conv0_W, conv0_b = f32(inputs["conv0_W"]), f32(inputs["conv0_b"])
    convs_W, convs_b = f32(inputs["convs_W"]), f32(inputs["convs_b"])

    sA, sB = _gate_row_scale()
    perms = [_PERM_A, _PERM_B]
    scales = [sA, sB]

    lw = np.zeros((128, LW_COLS), np.float32)

    def cvec(v, c):  # [256] -> permuted/scaled [128]
        return _chunk(v[:, None], perms[c], scales[c])[:, 0]

    for c in range(2):
        p, s = perms[c], scales[c]
        bb = _blk(_b_encl0(c))
        # ENC_BIAS rows 0:5
        for j in range(L):
            lw[j, bb] = cvec(enc_b[j], c)
        # ENC_L0REC rows 64:128
        lw[64:128, bb] = (_chunk(enc_Whh[0], p, s) * 0.5).T
        bx = _blk(_b_encx(c))
        # ENC_X row 0 (x input: no 0.5)
        lw[0, bx] = _chunk(enc_Wih0, p, s)[:, 0]
        # DEC_FCIN rows 64:128 : outer(0.5*fc_W, dec_Wih0_chunk)
        lw[64:128, bx] = np.outer(0.5 * fc_W[0], _chunk(dec_Wih0, p, s)[:, 0])
        # ENC_KST layers 1..4
        for l in range(1, L):
            ci = _chunk(enc_Wih[l - 1], p, s) * 0.5
            ch = _chunk(enc_Whh[l], p, s) * 0.5
            lw[:, _blk(_b_kst(l, c))] = np.concatenate([ci.T, ch.T], axis=0)
        # DEC_REC blocks + small tenants
        for l in range(L):
            br = _blk(_b_decrec(l, c))
            lw[64:128, br] = (_chunk(dec_Whh[l], p, s) * 0.5).T
        lw[0, _blk(4 + c)] = _chunk(dec_Wih0, p, s)[:, 0]          # DEC_X
        lw[0, _blk(_b_decbias(0, c, k0=True))] = cvec(dec_b[0], c)
        lw[0, _blk(_b_decbias(0, c))] = cvec(
            dec_b[0] + dec_Wih0[:, 0] * fc_b[0], c)
        for l in range(1, L):
            lw[0, _blk(_b_decbias(l, c))] = cvec(dec_b[l], c)
        # DEC_IN layers 1..4
        for l in range(1, L):
            lw[64:128, _blk(_b_decin(l, c))] = (
                _chunk(dec_Wih[l - 1], p, s) * 0.5).T
    lw[64:128, FC_COL] = 0.5 * fc_W[0]
    lw = lw.astype(BF)

    # ---- cnnw: conv1-7 tap-pair stationaries + conv0 4x4 folded ----
    cb = []
    for i in range(CNN_LAYERS - 1):
        for pp in range(6):
            blk = np.zeros((128, 64), np.float32)
            if pp < 3:
                dy = pp - 1
                blk[0:64] = convs_W[i, :, :, dy + 1, 0].T
                blk[64:128] = convs_W[i, :, :, dy + 1, 1].T
            else:
                dy = pp - 4
                blk[0:64] = convs_W[i, :, :, dy + 1, 2].T
            cb.append(blk)
    # conv0 folded with avgpool: 4x4 stride-2 kernel, 16 taps
    W4 = np.zeros((16, 64), np.float32)
    for u in range(4):
        for v in range(4):
            acc = np.zeros(64, np.float32)
            for a in range(2):
                for b2 in range(2):
                    if 0 <= u - a <= 2 and 0 <= v - b2 <= 2:
                        acc += 0.25 * conv0_W[:, 0, u - a, v - b2]
            W4[u * 4 + v] = acc
    c0blk = np.zeros((128, 64), np.float32)
    c0blk[0:16] = W4
    cb.append(c0blk)
    cnnw = np.concatenate(cb, axis=1).astype(BF)   # [128, 42*64+64=2752]
    C0_COL = (CNN_LAYERS - 1) * 6 * 64             # 2688

    # ---- indc: [5, GW5] layer indicators ----
    indc = np.zeros((5, GW5), np.float32)
    for j in range(L):
        indc[j, j * GW:(j + 1) * GW] = 1.0
    indc = indc.astype(BF)

    # ---- ones row (for K=1 bias matmuls at partition 0) ----
    ones = np.ones((1, GW), np.float32).astype(BF)

    # ---- misc f32 [64, 16]: conv/fc biases ----
    misc = np.zeros((64, 16), np.float32)
    misc[:, 0] = conv0_b
    for i in range(CNN_LAYERS - 1):
        misc[:, 1 + i] = convs_b[i]
    misc[0, 8] = fc_b[0]

    # ---- per-core tensors ----
    ypad = np.pad(y[:, 0], ((0, 0), (1, 1), (1, 1)))  # [B, 34, 34]
    in_maps = []
    for cidx in range(NCORES):
        sl = slice(cidx * BP, (cidx + 1) * BP)
        xs = x[sl, :, 0]  # [BP, T]
        xtm = np.ascontiguousarray(xs.T).reshape(1, T * BP).astype(BF)
        yp = ypad[sl]  # [BP, 34, 34]
        # conv0 4x4/stride2 im2col: [16, BP*256]
        yim = np.zeros((16, BP, PM, PM), np.float32)
        for u in range(4):
            for v in range(4):
                yim[u * 4 + v] = yp[:, u:u + 32:2, v:v + 32:2]
        yim = yim.reshape(16, BP * PM * PM).astype(BF)
        in_maps.append(dict(
            lstmw=lw, cnnw=cnnw, indc=indc, ones=ones, misc=misc,
            x=xtm, yim=yim,
        ))
    return in_maps, C0_COL


# ----------------------------------------------------------------------------
# device program
# ----------------------------------------------------------------------------

_CACHE = {}


def build_program(C0_COL):
    import concourse.bass as bass  # noqa: F401
    import concourse.tile as tile
    from concourse import bacc, mybir

    F32 = mybir.dt.float32
    BF16 = mybir.dt.bfloat16
    AF = mybir.ActivationFunctionType
    OP = mybir.AluOpType

    TICKS = int(os.environ.get("BASSK_TICKS", T + L - 1))  # 260
    DSTEPS = int(os.environ.get("BASSK_DSTEPS", PS))
    DO_CNN = int(os.environ.get("BASSK_CNN", 1))
    NCONV = int(os.environ.get("BASSK_NCONV", CNN_LAYERS))
    CNN_EVERY = int(os.environ.get("BASSK_CNN_EVERY", 3))

    nc = bacc.Bacc("TRN2", target_bir_lowering=False, debug=False,
                   num_devices=NCORES)
    d_lstmw = nc.dram_tensor("lstmw", [128, LW_COLS], BF16,
                             kind="ExternalInput").ap()
    d_cnnw = nc.dram_tensor("cnnw", [128, 2752], BF16,
                            kind="ExternalInput").ap()
    d_indc = nc.dram_tensor("indc", [5, GW5], BF16, kind="ExternalInput").ap()
    d_ones = nc.dram_tensor("ones", [1, GW], BF16, kind="ExternalInput").ap()
    d_misc = nc.dram_tensor("misc", [64, 16], F32, kind="ExternalInput").ap()
    d_x = nc.dram_tensor("x", [1, T * BP], BF16, kind="ExternalInput").ap()
    d_yim = nc.dram_tensor("yim", [16, BP * PM * PM], BF16,
                           kind="ExternalInput").ap()
    d_out = nc.dram_tensor("out", [1, PS * BP], F32, kind="ExternalOutput").ap()

    with tile.TileContext(nc) as tc:
        with ExitStack() as ctx:
            const = ctx.enter_context(tc.tile_pool(name="const", bufs=1))
            state = ctx.enter_context(tc.tile_pool(name="state", bufs=1))
            spool = ctx.enter_context(tc.tile_pool(name="spool", bufs=2))
            mpool = ctx.enter_context(tc.tile_pool(name="mpool", bufs=2))
            dpool = ctx.enter_context(tc.tile_pool(name="dpool", bufs=2))
            eps = ctx.enter_context(
                tc.tile_pool(name="eps", bufs=NG, space="PSUM"))
            cps = ctx.enter_context(tc.tile_pool(name="cps", bufs=2,
                                                 space="PSUM"))
            dps = ctx.enter_context(tc.tile_pool(name="dps", bufs=3,
                                                 space="PSUM"))
            fps = ctx.enter_context(tc.tile_pool(name="fps", bufs=1,
                                                 space="PSUM"))

            # ---- constants ----
            lw = const.tile([128, LW_COLS], BF16, tag="lw", name="lw")
            nc.sync.dma_start(lw[:], d_lstmw)
            cw = const.tile([128, 2752], BF16, tag="cw", name="cw")
            if DO_CNN:
                nc.sync.dma_start(cw[:], d_cnnw)
            indct = const.tile([5, GW5], BF16, tag="indct", name="indct")
            nc.sync.dma_start(indct[:], d_indc)
            onest = const.tile([1, GW], BF16, tag="onest", name="onest")
            nc.sync.dma_start(onest[:], d_ones)
            misct = const.tile([64, 16], F32, tag="misct", name="misct")
            nc.sync.dma_start(misct[:], d_misc)
            xw = const.tile([1, T * BP], BF16, tag="xw", name="xw")
            nc.sync.dma_start(xw[:], d_x)
            yimt = const.tile([16, BP * PM * PM], BF16, tag="yimt",
                              name="yimt")
            if DO_CNN:
                nc.sync.dma_start(yimt[:], d_yim)

            # ---- persistent state ----
            Ht = state.tile([128, W5], BF16, tag="H", name="H")
            Ct = state.tile([64, W5], F32, tag="C", name="C")
            nc.gpsimd.memset(Ht[:], 0.0)
            nc.gpsimd.memset(Ct[:], 0.0)
            z2a = state.tile([128, BP * PIMG], BF16, tag="z2a", name="z2a")
            z2b = state.tile([128, BP * PIMG], BF16, tag="z2b", name="z2b")
            if DO_CNN:
                nc.gpsimd.memset(z2a[:], 0.0)
                nc.gpsimd.memset(z2b[:], 0.0)
            feat = state.tile([64, BP], F32, tag="feat", name="feat")
            feat2 = state.tile([128, BP], BF16, tag="feat2", name="feat2")
            outt = state.tile([1, PS * BP], F32, tag="outt", name="outt")
            if DSTEPS == 0 or int(os.environ.get("BASSK_NOFC", 0)):
                nc.gpsimd.memset(outt[:], 0.0)
            if not DO_CNN:
                nc.gpsimd.memset(feat2[:], 0.0)

            Htv = Ht[:].rearrange("p (l b) -> p l b", l=L)
            Ctv = Ct[:].rearrange("p (l b) -> p l b", l=L)
            indcv = indct[:].rearrange("p (l b) -> p l b", l=L)

            # ============ CNN thunks (interleaved into encoder) ============
            cnn_thunks = []
            if DO_CNN:
                z1v = z2a[:].rearrange("p (i r c) -> p i r c", i=BP, r=PPAD)

                def conv0_chunk(n):
                    def f():
                        pc = cps.tile([64, 512], F32, tag="cpg", name="cpg")
                        nc.tensor.matmul(
                            pc[:], cw[0:16, C0_COL:C0_COL + 64],
                            yimt[0:16, n * 512:(n + 1) * 512],
                            start=True, stop=True)
                        nc.scalar.activation(
                            z1v[0:64, 2 * n:2 * n + 2, 1:17, 1:17],
                            pc[:].rearrange("p (i r c) -> p i r c", i=2, r=16),
                            AF.Identity, bias=misct[0:64, 0:1])
                        nc.gpsimd.tensor_copy(
                            z1v[64:128, 2 * n:2 * n + 2, 1:17, 0:16],
                            z1v[0:64, 2 * n:2 * n + 2, 1:17, 1:17])
                    return f

                for n in range(BP // 2):
                    cnn_thunks.append(conv0_chunk(n))

                def conv_group(i, n, zin, zout):
                    def f():
                        ziv = zin[:].rearrange("p (i r c) -> p i r c",
                                               i=BP, r=PPAD)
                        zov = zout[:].rearrange("p (i r c) -> p i r c",
                                                i=BP, r=PPAD)
                        i0 = 2 * n
                        pc = cps.tile([64, 512], F32, tag="cpg", name="cpg")
                        for pp in range(6):
                            dy = (pp - 1) if pp < 3 else (pp - 4)
                            c0_ = 0 if pp < 3 else 2
                            st_ = cw[:, (i - 1) * 384 + pp * 64:
                                     (i - 1) * 384 + pp * 64 + 64]
                            rhs = ziv[:, i0:i0 + 2, 1 + dy:17 + dy,
                                      c0_:c0_ + 16]
                            nc.tensor.matmul(pc[:], st_, rhs,
                                             start=(pp == 0), stop=(pp == 5))
                        nc.scalar.activation(
                            zov[0:64, i0:i0 + 2, 1:17, 1:17],
                            pc[:].rearrange("p (i r c) -> p i r c", i=2, r=16),
                            AF.Relu, bias=misct[0:64, i:i + 1])
                        if i < CNN_LAYERS - 1:
                            nc.gpsimd.tensor_copy(
                                zov[64:128, i0:i0 + 2, 1:17, 0:16],
                                zov[0:64, i0:i0 + 2, 1:17, 1:17])
                    return f

                zin, zout = z2a, z2b
                for i in range(1, NCONV):
                    for n in range(BP // 2):
                        cnn_thunks.append(conv_group(i, n, zin, zout))
                    zin, zout = zout, zin

                def gap_thunk(zfin):
                    def f():
                        zfv = zfin[:].rearrange("p (i r c) -> p i r c",
                                                i=BP, r=PPAD)
                        for j in range(BP):
                            nc.vector.tensor_reduce(
                                feat[:, j:j + 1], zfv[0:64, j, 1:17, 1:17],
                                axis=mybir.AxisListType.XY, op=OP.add)
                        nc.vector.tensor_copy(feat2[64:128, :], feat[:])
                    return f

                cnn_thunks.append(gap_thunk(zin))
            cnn_iter = iter(cnn_thunks)
            sub_idx = 0

            # =============== encoder wavefront ===============
            for s in range(TICKS):
                lmin = max(0, s - (T - 1))
                lmax = min(L - 1, s)
                sl = slice(lmin, lmax + 1)
                for g in range(NG):
                    gsl = slice(g * GW, (g + 1) * GW)
                    pg = eps.tile([128, 2 * GW5], F32, tag="epg", name="epg")
                    pgv = pg[:].rearrange("p (c l b) -> p c l b", c=2, l=L)
                    for c in range(2):
                        nc.tensor.matmul(
                            pgv[:, c, sl, :], lw[0:5, _blk(_b_encl0(c))],
                            indcv[:, sl, :], start=True, stop=False)
                        if lmin == 0:
                            nc.tensor.matmul(
                                pgv[:, c, 0, :], lw[0:1, _blk(_b_encx(c))],
                                xw[0:1, s * BP + g * GW:s * BP + g * GW + GW],
                                start=False, stop=False)
                            nc.tensor.matmul(
                                pgv[:, c, 0, :], lw[64:128, _blk(_b_encl0(c))],
                                Htv[64:128, 0, gsl],
                                start=False, stop=(lmax == 0))
                        for l in range(max(1, lmin), lmax + 1):
                            nc.tensor.matmul(
                                pgv[:, c, l, :], lw[:, _blk(_b_kst(l, c))],
                                Htv[:, l, gsl],
                                start=False, stop=(l == lmax))
                    st = spool.tile([128, 2 * GW5], F32, tag=f"st{g}",
                                    name="st")
                    stv = st[:].rearrange("p (c l b) -> p c l b", c=2, l=L)
                    nc.scalar.activation(stv[:, :, sl, :], pgv[:, :, sl, :],
                                         AF.Tanh, scale=0.5)
                    m1 = mpool.tile([64, GW5], F32, tag=f"m1{g}", name="m1")
                    m2 = mpool.tile([64, GW5], F32, tag=f"m2{g}", name="m2")
                    tcn = mpool.tile([64, GW5], F32, tag=f"tc{g}", name="tcn")
                    m1v = m1[:].rearrange("p (l b) -> p l b", l=L)
                    m2v = m2[:].rearrange("p (l b) -> p l b", l=L)
                    tcv = tcn[:].rearrange("p (l b) -> p l b", l=L)
                    nc.vector.scalar_tensor_tensor(
                        m1v[:, sl, :], stv[0:64, 0, sl, :], 1.0,
                        Ctv[:, sl, gsl], op0=OP.add, op1=OP.mult)
                    nc.vector.scalar_tensor_tensor(
                        m2v[:, sl, :], stv[64:128, 0, sl, :], 1.0,
                        stv[64:128, 1, sl, :], op0=OP.add, op1=OP.mult)
                    nc.vector.scalar_tensor_tensor(
                        Ctv[:, sl, gsl], m1v[:, sl, :], 0.5,
                        m2v[:, sl, :], op0=OP.mult, op1=OP.add)
                    nc.scalar.activation(tcv[:, sl, :], Ctv[:, sl, gsl],
                                         AF.Tanh, scale=0.5)
                    nc.vector.scalar_tensor_tensor(
                        Htv[64:128, sl, gsl], stv[0:64, 1, sl, :], 1.0,
                        tcv[:, sl, :], op0=OP.add, op1=OP.mult)
                    # shift-copy for next tick: top[l] = bot[l-1] (on Pool)
                    if s + 1 < TICKS:
                        nlmax = min(L - 1, s + 1)
                        a = max(1, max(0, s + 1 - (T - 1)))
                        if nlmax >= 1:
                            nc.vector.tensor_copy(
                                Htv[0:64, a:nlmax + 1, gsl],
                                Htv[64:128, a - 1:nlmax, gsl])
                    sub_idx += 1
                    if sub_idx % CNN_EVERY == 0:
                        th = next(cnn_iter, None)
                        if th is not None:
                            th()
            for th in cnn_iter:
                th()

            # =============== fuse -> decoder init ===============
            kf = 2.0 * ALPHA / 256.0
            for l in range(L):
                nc.vector.scalar_tensor_tensor(
                    Htv[64:128, l, :], feat2[64:128, :], kf,
                    Htv[64:128, l, :], op0=OP.mult, op1=OP.add)

            # =============== decoder ===============
            for k in range(DSTEPS):
                for g in range(NG):
                    gsl = slice(g * GW, (g + 1) * GW)
                    for l in range(L):
                        pd = dps.tile([128, 2 * GW], F32, tag="dpg",
                                      name="dpg")
                        pdv = pd[:].rearrange("p (c b) -> p c b", c=2)
                        for c in range(2):
                            # group order: p0 matmuls first, then p64
                            bias_ap = lw[0:1, _blk(_b_decbias(l, c, k == 0))]
                            nc.tensor.matmul(pdv[:, c, :], bias_ap,
                                             onest[0:1, :],
                                             start=True, stop=False)
                            if l == 0 and k == 0:
                                nc.tensor.matmul(
                                    pdv[:, c, :], lw[0:1, _blk(4 + c)],
                                    xw[0:1, (T - 1) * BP + g * GW:
                                       (T - 1) * BP + g * GW + GW],
                                    start=False, stop=False)
                            nc.tensor.matmul(
                                pdv[:, c, :], lw[64:128, _blk(_b_decrec(l, c))],
                                Htv[64:128, l, gsl], start=False,
                                stop=(l == 0 and k == 0))
                            if l == 0:
                                if k > 0:
                                    nc.tensor.matmul(
                                        pdv[:, c, :],
                                        lw[64:128, _blk(_b_encx(c))],
                                        Htv[64:128, L - 1, gsl],
                                        start=False, stop=True)
                            else:
                                nc.tensor.matmul(
                                    pdv[:, c, :],
                                    lw[64:128, _blk(_b_decin(l, c))],
                                    Htv[64:128, l - 1, gsl],
                                    start=False, stop=True)
                        sd = dpool.tile([128, 2 * GW], F32, tag=f"sd{g}",
                                        name="sd")
                        sdv = sd[:].rearrange("p (c b) -> p c b", c=2)
                        nc.scalar.activation(sd[:], pd[:], AF.Tanh, scale=0.5)
                        dm1 = dpool.tile([64, GW], F32, tag=f"dm1{g}",
                                         name="dm1")
                        dm2 = dpool.tile([64, GW], F32, tag=f"dm2{g}",
                                         name="dm2")
                        dtc = dpool.tile([64, GW], F32, tag=f"dtc{g}",
                                         name="dtc")
                        nc.vector.scalar_tensor_tensor(
                            dm1[:], sdv[0:64, 0, :], 1.0, Ctv[:, l, gsl],
                            op0=OP.add, op1=OP.mult)
                        nc.vector.scalar_tensor_tensor(
                            dm2[:], sdv[64:128, 0, :], 1.0,
                            sdv[64:128, 1, :], op0=OP.add, op1=OP.mult)
                        nc.vector.scalar_tensor_tensor(
                            Ctv[:, l, gsl], dm1[:], 0.5, dm2[:],
                            op0=OP.mult, op1=OP.add)
                        nc.scalar.activation(dtc[:], Ctv[:, l, gsl], AF.Tanh,
                                             scale=0.5)
                        nc.vector.scalar_tensor_tensor(
                            Htv[64:128, l, gsl], sdv[0:64, 1, :], 1.0,
                            dtc[:], op0=OP.add, op1=OP.mult)
                    # fc + output (off the AR chain)
                    if int(os.environ.get("BASSK_NOFC", 0)) == 0:
                        pf = fps.tile([1, GW], F32, tag="fpg", name="fpg")
                        nc.tensor.matmul(pf[:], lw[64:128, FC_COL:FC_COL + 1],
                                         Htv[64:128, L - 1, gsl],
                                         start=True, stop=True)
                        nc.scalar.activation(
                            outt[0:1, k * BP + g * GW:k * BP + (g + 1) * GW],
                            pf[:], AF.Identity, bias=misct[0:1, 8:9])

            nc.sync.dma_start(d_out, outt[:])

    nc.compile()
    return nc


def kernel(**inputs) -> np.ndarray:
    from concourse.bass_utils import run_bass_kernel_spmd
    in_maps, c0col = prep_host(inputs)
    if "nc" not in _CACHE:
        _CACHE["nc"] = build_program(c0col)
    nc = _CACHE["nc"]
    res = run_bass_kernel_spmd(nc, in_maps, list(range(NCORES)))
    outs = []
    for c in range(NCORES):
        o = np.asarray(res.results[c]["out"], np.float32).reshape(PS, BP)
        outs.append(o.T[:, :, None])  # [BP, PS, 1]
    return np.concatenate(outs, axis=0)
